# revision 6
# baseline (speedup 1.0000x reference)
"""Non-local block (B=4, C_in=256, C_int=128, C_out=256, N=T*H*W=4096) on 8
Trainium2 NeuronCores.

Sharding: data-parallel over batch (4 batches) x query-halves (2) = 8 cores.
Each core holds one batch's full x (for keys/values) plus its query half,
computes theta/phi/g projections, the [2048q x 4096k] attention with softmax
(keys on partitions; denominator via an all-ones stationary matmul;
normalization applied to y after the attn @ g contraction), and the output
projection for its query half. Host gathers the 8 [256, 2048] slices.
"""

import numpy as np

import concourse.bacc as bacc
import concourse.bass as bass
import concourse.mybir as mybir
import concourse.tile as tile
from concourse.bass_utils import run_bass_kernel_spmd
from concourse.masks import make_identity

F32 = mybir.dt.float32
F32R = mybir.dt.float32r
AF = mybir.ActivationFunctionType
OP = mybir.AluOpType

P = 128
CI = 256  # input channels (2 chunks of 128)
CINT = 128  # intermediate channels
CO = 256  # output channels (2 blocks of 128)
N = 4096  # key/value positions (32 blocks of 128)
Q = 2048  # queries per core
B, T, H, W = 4, 4, 32, 32
NKB = N // P  # 32 key blocks
QG = 1024  # query group width (2 groups per core)
NQG = Q // QG

# dtype used for matmul operands (fp32 data bitcast to f32r runs the PE at
# full rate for free dims >= 256; plain float32 runs at 1/4 rate)
MM_DT = F32R


def build():
    nc = bacc.Bacc(None, target_bir_lowering=False, debug=False)

    xb = nc.dram_tensor("xb", [CI, N], F32, kind="ExternalInput").ap()
    xq = nc.dram_tensor("xq", [CI, Q], F32, kind="ExternalInput").ap()
    wt = nc.dram_tensor("wt", [CINT, CI], F32, kind="ExternalInput").ap()
    wp = nc.dram_tensor("wp", [CINT, CI], F32, kind="ExternalInput").ap()
    wg = nc.dram_tensor("wg", [CINT, CI], F32, kind="ExternalInput").ap()
    wo = nc.dram_tensor("wo", [CO, CINT], F32, kind="ExternalInput").ap()
    bt = nc.dram_tensor("bt", [CINT], F32, kind="ExternalInput").ap()
    bp = nc.dram_tensor("bp", [CINT], F32, kind="ExternalInput").ap()
    bg = nc.dram_tensor("bg", [CINT], F32, kind="ExternalInput").ap()
    bo = nc.dram_tensor("bo", [CO], F32, kind="ExternalInput").ap()
    oq = nc.dram_tensor("oq", [CO, Q], F32, kind="ExternalOutput").ap()

    with tile.TileContext(nc) as tc:
        with (
            tc.tile_pool(name="consts", bufs=1) as consts,
            tc.tile_pool(name="big", bufs=1) as big,
            tc.tile_pool(name="tmp", bufs=3) as tmp,
        ):
            # ---- constants / weights into SBUF ----
            wt_sb = consts.tile([P, CI], F32, tag="wt")
            wp_sb = consts.tile([P, CI], F32, tag="wp")
            wg_sb = consts.tile([P, CI], F32, tag="wg")
            wo_sb = consts.tile([P, 2, CINT], F32, tag="wo")
            nc.sync.dma_start(wt_sb[:], wt)
            nc.sync.dma_start(wp_sb[:], wp)
            nc.sync.dma_start(wg_sb[:], wg)
            nc.sync.dma_start(wo_sb[:], wo.rearrange("(o p) c -> p o c", p=P))

            bt_sb = consts.tile([P, 1], F32, tag="bt")
            bp_sb = consts.tile([P, 1], F32, tag="bp")
            bo_sb = consts.tile([P, 2], F32, tag="bo")
            nc.sync.dma_start(bt_sb[:], bt[:, None])
            nc.sync.dma_start(bp_sb[:], bp[:, None])
            nc.sync.dma_start(bo_sb[:], bo.rearrange("(o p) -> p o", p=P))
            # b_g broadcast along partitions: every partition row holds b_g
            bg_bcast = consts.tile([P, CINT], F32, tag="bgb")
            bg_b_ap = bass.AP(tensor=bg.tensor, offset=bg.offset, ap=[[0, P], [1, CINT]])
            nc.gpsimd.dma_start(out=bg_bcast[:], in_=bg_b_ap)

            identity = consts.tile([P, P], F32, tag="ident")
            make_identity(nc, identity[:])
            ones_f32 = consts.tile([P, P], F32, tag="ones_f")
            nc.vector.memset(ones_f32[:], 1.0)
            ones_sb = consts.tile([P, P], MM_DT, tag="ones")
            nc.vector.tensor_copy(out=ones_sb[:], in_=ones_f32[:])

            # ---- x into SBUF (chunked so compute can start early) ----
            xq_sb = big.tile([P, 2, Q], MM_DT, tag="xq")
            xqr = xq.rearrange("(o p) q -> p o q", p=P).bitcast(MM_DT)
            for j in range(Q // 512):
                sl = slice(j * 512, (j + 1) * 512)
                nc.sync.dma_start(xq_sb[:, :, sl], xqr[:, :, sl])
            x_sb = big.tile([P, 2, N], MM_DT, tag="x")
            xbr = xb.rearrange("(o p) n -> p o n", p=P).bitcast(MM_DT)
            for j in range(N // 512):
                sl = slice(j * 512, (j + 1) * 512)
                nc.sync.dma_start(x_sb[:, :, sl], xbr[:, :, sl])

            # ---- transpose projection weights on the PE ----
            # wT[ci_inner, o, co] = w[co, o*128 + ci_inner]
            with tc.tile_pool(name="ps_w", bufs=2, space="PSUM") as ps_w:
                wtT = consts.tile([P, 2, CINT], MM_DT, tag="wtT")
                wpT = consts.tile([P, 2, CINT], MM_DT, tag="wpT")
                wgT = consts.tile([P, 2, CINT], MM_DT, tag="wgT")
                woT = consts.tile([P, 2, P], MM_DT, tag="woT")
                for w_sb, wT in ((wt_sb, wtT), (wp_sb, wpT), (wg_sb, wgT)):
                    for o in range(2):
                        pw = ps_w.tile([P, P], F32, tag="pw")
                        nc.tensor.transpose(
                            pw[:], w_sb[:, o * P : (o + 1) * P], identity[:]
                        )
                        nc.vector.tensor_copy(out=wT[:, o, :], in_=pw[:])
                for blk in range(2):
                    pw = ps_w.tile([P, P], F32, tag="pw")
                    nc.tensor.transpose(pw[:], wo_sb[:, blk, :], identity[:])
                    nc.vector.tensor_copy(out=woT[:, blk, :], in_=pw[:])

            # ---- projections ----
            theta_sb = big.tile([P, Q], MM_DT, tag="theta")
            phi_sb = big.tile([P, N], MM_DT, tag="phi")
            gT_sb = big.tile([P, NKB, P], MM_DT, tag="gT")

            with (
                tc.tile_pool(name="ps_proj", bufs=2, space="PSUM") as ps_proj,
                tc.tile_pool(name="ps_g", bufs=2, space="PSUM") as ps_g,
            ):
                for j in range(Q // 512):
                    sl = slice(j * 512, (j + 1) * 512)
                    pp = ps_proj.tile([P, 512], F32, tag="pp")
                    nc.tensor.matmul(
                        pp[:], wtT[:, 0, :], xq_sb[:, 0, sl],
                        start=True, stop=False,
                    )
                    nc.tensor.matmul(
                        pp[:], wtT[:, 1, :], xq_sb[:, 1, sl],
                        start=False, stop=True,
                    )
                    nc.vector.tensor_scalar(
                        out=theta_sb[:, sl], in0=pp[:], scalar1=bt_sb[:],
                        scalar2=None, op0=OP.add,
                    )
                for j in range(N // 512):
                    sl = slice(j * 512, (j + 1) * 512)
                    pp = ps_proj.tile([P, 512], F32, tag="pp")
                    nc.tensor.matmul(
                        pp[:], wpT[:, 0, :], x_sb[:, 0, sl],
                        start=True, stop=False,
                    )
                    nc.tensor.matmul(
                        pp[:], wpT[:, 1, :], x_sb[:, 1, sl],
                        start=False, stop=True,
                    )
                    nc.vector.tensor_scalar(
                        out=phi_sb[:, sl], in0=pp[:], scalar1=bp_sb[:],
                        scalar2=None, op0=OP.add,
                    )
                # g, transposed: gT[k, c] = sum_ci x[ci, k] * wg[c, ci] + bg[c]
                for kb in range(NKB):
                    ksl = slice(kb * P, (kb + 1) * P)
                    pg = ps_g.tile([P, P], F32, tag="pg")
                    nc.tensor.matmul(
                        pg[:], x_sb[:, 0, ksl], wgT[:, 0, :],
                        start=True, stop=False,
                    )
                    nc.tensor.matmul(
                        pg[:], x_sb[:, 1, ksl], wgT[:, 1, :],
                        start=False, stop=True,
                    )
                    nc.vector.tensor_tensor(
                        out=gT_sb[:, kb, :], in0=pg[:], in1=bg_bcast[:], op=OP.add
                    )

            # ---- attention (keys on partitions), software-pipelined ----
            y_sb = big.tile([P, Q], MM_DT, tag="y")
            with (
                tc.tile_pool(name="ps_s", bufs=2, space="PSUM") as ps_s,
                tc.tile_pool(name="ps_acc", bufs=1, space="PSUM") as ps_acc,
            ):
                for qg in range(NQG):
                    qsl = slice(qg * QG, (qg + 1) * QG)
                    y_ps = ps_acc.tile([P, QG], F32, tag="y_ps")
                    d_ps = ps_acc.tile([P, QG], F32, tag="d_ps")

                    def scores(kb):
                        s_ps = ps_s.tile([P, QG], F32, tag="s_ps")
                        for h in range(QG // 512):
                            nc.tensor.matmul(
                                s_ps[:, h * 512 : (h + 1) * 512],
                                phi_sb[:, kb * P : (kb + 1) * P],
                                theta_sb[:, qg * QG + h * 512 : qg * QG + (h + 1) * 512],
                                start=True, stop=True,
                            )
                        return s_ps

                    s_cur = scores(0)
                    for kb in range(NKB):
                        at = tmp.tile([P, QG], MM_DT, tag="attn")
                        nc.scalar.activation(out=at[:], in_=s_cur[:], func=AF.Exp)
                        if kb + 1 < NKB:
                            s_cur = scores(kb + 1)
                        first, last = kb == 0, kb == NKB - 1
                        for h in range(QG // 512):
                            hsl = slice(h * 512, (h + 1) * 512)
                            nc.tensor.matmul(
                                y_ps[:, hsl], gT_sb[:, kb, :],
                                at[:, hsl], start=first, stop=last,
                            )
                            nc.tensor.matmul(
                                d_ps[:, hsl], ones_sb[:],
                                at[:, hsl], start=first, stop=last,
                            )
                    rd = tmp.tile([P, QG], F32, tag="rd")
                    nc.vector.reciprocal(out=rd[:], in_=d_ps[:])
                    nc.vector.tensor_tensor(
                        out=y_sb[:, qsl], in0=y_ps[:], in1=rd[:], op=OP.mult
                    )

            # ---- output projection ----
            out_sb = big.tile([P, 2, Q], F32, tag="out")
            oqr = oq.rearrange("(o p) q -> p o q", p=P)
            with tc.tile_pool(name="ps_out", bufs=2, space="PSUM") as ps_out:
                for j in range(Q // 512):
                    sl = slice(j * 512, (j + 1) * 512)
                    for blk in range(2):
                        po = ps_out.tile([P, 512], F32, tag="po")
                        nc.tensor.matmul(
                            po[:], woT[:, blk, :], y_sb[:, sl],
                            start=True, stop=True,
                        )
                        nc.vector.tensor_scalar(
                            out=out_sb[:, blk, sl], in0=po[:],
                            scalar1=bo_sb[:, blk : blk + 1], scalar2=None, op0=OP.add,
                        )
                    nc.sync.dma_start(oqr[:, :, sl], out_sb[:, :, sl])

    nc.compile()
    return nc


_NC_CACHE = None
LAST_EXEC_TIME_NS = None
LAST_TRACE = None


def _get_nc():
    global _NC_CACHE
    if _NC_CACHE is None:
        _NC_CACHE = build()
    return _NC_CACHE


def kernel(**inputs):
    x = np.ascontiguousarray(np.asarray(inputs["x"], dtype=np.float32))
    assert x.shape == (B, CI, T, H, W), x.shape
    xf = x.reshape(B, CI, N)
    w = {
        k: np.ascontiguousarray(np.asarray(inputs[k], dtype=np.float32))
        for k in (
            "w_theta", "b_theta", "w_phi", "b_phi", "w_g", "b_g", "w_out", "b_out"
        )
    }

    in_maps = []
    for core in range(8):
        b, h = core // 2, core % 2
        in_maps.append(
            {
                "xb": xf[b],
                "xq": np.ascontiguousarray(xf[b][:, h * Q : (h + 1) * Q]),
                "wt": w["w_theta"], "wp": w["w_phi"], "wg": w["w_g"],
                "wo": w["w_out"], "bt": w["b_theta"], "bp": w["b_phi"],
                "bg": w["b_g"], "bo": w["b_out"],
            }
        )

    nc = _get_nc()
    res = run_bass_kernel_spmd(nc, in_maps, core_ids=list(range(8)))
    global LAST_EXEC_TIME_NS, LAST_TRACE
    LAST_EXEC_TIME_NS = res.exec_time_ns
    LAST_TRACE = res.instructions_and_trace[1] if res.instructions_and_trace else None

    out = np.empty((B, CO, N), np.float32)
    for core in range(8):
        b, h = core // 2, core % 2
        out[b][:, h * Q : (h + 1) * Q] = res.results[core]["oq"]
    return out.reshape(B, CO, T, H, W)
